# revision 9
# baseline (speedup 1.0000x reference)
"""Causal multi-head attention block (B=4, S=2048, NX=1024, H=16, D=64)
distributed over 8 TRN2 NeuronCores.

Sharding: core i handles batch b = i//2 and head-group hg = i%2 (8 of 16
heads).  Each core computes qkv for its heads, causal attention, and a
partial c_proj over its 512 feature rows; the per-batch pair of cores
reduces partials with an on-chip ReduceScatter (or on the host).

All matmuls run in bf16 (f32 PSUM accumulate).  Scores are computed in the
transposed orientation s^T[k, q] = k @ q^T; exp(s^T) tiles then serve as
the stationary operand of u = p @ v_aug (ones-augmented v), which puts the
softmax denominator in a per-partition column -> cheap reciprocal and
per-partition scaling.  A PE-transpose pass rebuilds a^T for c_proj.
"""
import sys

sys.path.insert(0, "/opt/trn_rl_repo")

import functools

import ml_dtypes
import numpy as np

from concourse import bacc, mybir, tile
from concourse.bass_utils import run_bass_kernel_spmd
from concourse.masks import make_identity

B, S, NX = 4, 2048, 1024
H, D = 16, 64
N_CORES = 8
HL = H // 2          # heads per core (local)
FL = HL * D          # local head feature width (512)
BF16 = mybir.dt.bfloat16
F32 = mybir.dt.float32
BF = ml_dtypes.bfloat16

NK = S // 128        # 16 k-tiles of 128
NQC = S // 512       # 4 q-chunks of 512
KK = NX // 128       # 8 contraction blocks

# "rs": on-chip ReduceScatter over core pairs; "host": partials summed on host
REDUCE_MODE = "rs"


def _build(reduce_mode: str):
    nc = bacc.Bacc("TRN2", target_bir_lowering=False, debug=False,
                   num_devices=N_CORES)

    xT_ext = nc.dram_tensor("xT", [NX, S], BF16, kind="ExternalInput")
    wqk_ext = nc.dram_tensor("w_qk", [NX, 2 * FL], BF16, kind="ExternalInput")
    wv_ext = nc.dram_tensor("w_v", [NX, FL], BF16, kind="ExternalInput")
    wp_ext = nc.dram_tensor("w_proj", [FL, NX], BF16, kind="ExternalInput")
    bqk_ext = nc.dram_tensor("b_qk", [2 * FL, 1], F32, kind="ExternalInput")
    bv_ext = nc.dram_tensor("bv_row", [1, FL], BF16, kind="ExternalInput")
    bp_ext = nc.dram_tensor("bp_row", [1, NX], BF16, kind="ExternalInput")
    if reduce_mode == "rs":
        out_ext = nc.dram_tensor("out", [S // 2, NX], F32, kind="ExternalOutput")
    else:
        out_ext = nc.dram_tensor("out", [S, NX], F32, kind="ExternalOutput")

    with tile.TileContext(nc) as tc:
        with tc.tile_pool(name="const", bufs=1) as cp, \
             tc.tile_pool(name="work", bufs=3) as wp, \
             tc.tile_pool(name="psS", bufs=2, space="PSUM") as psS, \
             tc.tile_pool(name="psU", bufs=2, space="PSUM") as psU, \
             tc.tile_pool(name="dram", bufs=1, space="DRAM") as dp:

            # ---- persistent SBUF tensors ----
            xT = cp.tile([128, KK, S], BF16, tag="xT")
            wqk = cp.tile([128, KK, 2 * FL], BF16, tag="wqk")
            wv = cp.tile([128, KK, FL], BF16, tag="wv")
            wproj = cp.tile([128, FL // 128, NX], BF16, tag="wproj")
            qkT = cp.tile([128, 2 * FL // 128, S], BF16, tag="qkT")
            v_aug = cp.tile([128, NK, HL, D + 1], BF16, tag="vaug")
            a_nat = cp.tile([128, NK, FL], BF16, tag="anat")   # a [q, feat]
            aT = cp.tile([128, FL // 128, S], BF16, tag="aT")  # a^T [feat, q]
            bqk = cp.tile([128, 2 * FL // 128], F32, tag="bqk")
            bv_row = cp.tile([1, FL], BF16, tag="bv")
            bp_row = cp.tile([1, NX], BF16, tag="bp")
            ones_row = cp.tile([1, 128], BF16, tag="ones")
            tri = cp.tile([128, 128], BF16, tag="tri")
            ident = cp.tile([128, 128], BF16, tag="ident")

            # ---- input DMAs ----
            for kk in range(KK):
                nc.sync.dma_start(out=xT[:, kk, :],
                                  in_=xT_ext.ap()[kk * 128:(kk + 1) * 128, :])
                nc.sync.dma_start(out=wqk[:, kk, :],
                                  in_=wqk_ext.ap()[kk * 128:(kk + 1) * 128, :])
                nc.sync.dma_start(out=wv[:, kk, :],
                                  in_=wv_ext.ap()[kk * 128:(kk + 1) * 128, :])
            for kt in range(FL // 128):
                nc.sync.dma_start(out=wproj[:, kt, :],
                                  in_=wp_ext.ap()[kt * 128:(kt + 1) * 128, :])
            for fb in range(2 * FL // 128):
                nc.sync.dma_start(out=bqk[:, fb:fb + 1],
                                  in_=bqk_ext.ap()[fb * 128:(fb + 1) * 128, :])
            nc.sync.dma_start(out=bv_row[:], in_=bv_ext.ap())
            nc.sync.dma_start(out=bp_row[:], in_=bp_ext.ap())

            nc.vector.memset(ones_row[:], 1.0)
            # tri[p, f] = 1 if p <= f else 0 (keep-in on p > f, else fill 1)
            nc.vector.memset(tri[:], 0.0)
            nc.gpsimd.affine_select(
                out=tri[:], in_=tri[:],
                compare_op=mybir.AluOpType.is_gt,
                fill=1.0, base=0, pattern=[[-1, 128]], channel_multiplier=1,
            )
            make_identity(nc, ident[:])
            # ones column of v_aug
            nc.vector.memset(v_aug[:, :, :, D:D + 1], 1.0)

            if reduce_mode == "rs":
                rs_in = dp.tile([S, NX], F32)
                rs_out = dp.tile([S // 2, NX], F32)

            # ---- stage 2: v (natural layout) ----
            def emit_v(st):
                ps = psS.tile([128, FL], F32, tag="ps")
                for kk in range(KK):
                    nc.tensor.matmul(ps[:], xT[:, kk, st * 128:(st + 1) * 128],
                                     wv[:, kk, :], start=(kk == 0), stop=False)
                nc.tensor.matmul(ps[:], ones_row[:], bv_row[:],
                                 start=False, stop=True)
                nc.vector.tensor_copy(
                    v_aug[:, st, :, 0:D],
                    ps[:].rearrange("p (h d) -> p h d", d=D))

            # ---- stage 1: q^T / k^T (feature-major) ----
            def emit_qk(fb):
                for n0, nw in ((0, 1536), (1536, 512)):
                    ps = psS.tile([128, nw], F32, tag="ps")
                    for c0 in range(0, nw, 512):
                        for kk in range(KK):
                            nc.tensor.matmul(
                                ps[:, c0:c0 + 512],
                                wqk[:, kk, fb * 128:(fb + 1) * 128],
                                xT[:, kk, n0 + c0:n0 + c0 + 512],
                                start=(kk == 0), stop=(kk == KK - 1))
                    nc.vector.tensor_scalar_add(qkT[:, fb, n0:n0 + nw], ps[:],
                                                bqk[:, fb:fb + 1])

            # ---- stage 3: attention for one (head, q-chunk) ----
            def emit_head_qc(lh, qc):
                fbq = lh // 2
                fbk = FL // 128 + lh // 2
                po = (lh % 2) * 64
                qb = qc * 512
                n_full = 4 * qc
                # groups of k-tiles: full tiles in 3s, then the 4 diagonal
                # tiles packed at PSUM-bank-safe offsets (no matmul output may
                # cross a 512-f32 bank boundary)
                groups = []
                kt0 = 0
                while kt0 < n_full:
                    g = min(3, n_full - kt0)
                    groups.append([(kt0 + j, j * 512, 512, 0) for j in range(g)])
                    kt0 += g
                diag_offs = (0, 512, 1024, 1280)
                groups.append([(n_full + j, diag_offs[j], 512 - 128 * j, 128 * j)
                               for j in range(4)])

                pu = psU.tile([128, 4, D + 1], F32, tag="pu")
                last_kt = n_full + 3
                for gi, g in enumerate(groups):
                    is_diag = gi == len(groups) - 1
                    gw = max(off + N for (_, off, N, _) in g)
                    ps = psS.tile([128, 1536], F32, tag="ps")
                    for (kt, off, N, qoff) in g:
                        nc.tensor.matmul(
                            ps[:, off:off + N],
                            qkT[po:po + 64, fbk, kt * 128:(kt + 1) * 128],
                            qkT[po:po + 64, fbq, qb + qoff:qb + 512],
                            start=True, stop=True)
                    p = wp.tile([128, 1536], BF16, tag="p")
                    nc.scalar.activation(p[:, 0:gw], ps[:, 0:gw],
                                         mybir.ActivationFunctionType.Exp,
                                         scale=0.125)
                    if is_diag:
                        for (kt, off, N, qoff) in g:
                            nc.vector.tensor_mul(p[:, off:off + 128],
                                                 p[:, off:off + 128], tri[:])
                    # u accumulation: p slices as stationary, v_aug moving.
                    # start=True clears has_written for the whole PSUM bank,
                    # so it must appear exactly once (first matmul of the
                    # tile); later first-writes of other regions overwrite
                    # because the bank-wide clear reset their has_written.
                    for (kt, off, N, qoff) in g:
                        j = kt - n_full  # >= 0 only for diag tiles
                        for qs in range(qoff // 128, 4):
                            pcol = off + qs * 128 - qoff
                            nc.tensor.matmul(
                                pu[:, qs, :],
                                p[:, pcol:pcol + 128],
                                v_aug[:, kt, lh, :],
                                start=(kt == 0 and qs == 0),
                                stop=(is_diag and j == 3),
                                skip_group_check=True)
                # normalize into a_nat[q, feat].  The reciprocal reads the
                # sumexp column of ALL 4 q-subtiles in one strided op so it
                # (and the muls that consume it) depend on every u-matmul —
                # a region-scoped read would be hoisted concurrent with PE
                # writes to the same PSUM bank and read torn data.
                recip4 = wp.tile([128, 4], F32, tag="recip")
                nc.vector.reciprocal(
                    recip4[:],
                    pu[:, :, D:D + 1].rearrange("p a b -> p (a b)"))
                for qs in range(4):
                    st = 4 * qc + qs
                    nc.vector.tensor_scalar_mul(
                        a_nat[:, st, lh * D:(lh + 1) * D],
                        pu[:, qs, 0:D], recip4[:, qs:qs + 1])

            # ---- stage 3.5 + 4: transpose a -> a^T, then c_proj partial ----
            def emit_proj(st):
                pt = psU.tile([128, 512], BF16, tag="pu")
                for fb in range(4):
                    nc.tensor.transpose(pt[:, fb * 128:(fb + 1) * 128],
                                        a_nat[:, st, fb * 128:(fb + 1) * 128],
                                        ident[:])
                nc.vector.tensor_copy(aT[:, :, st * 128:(st + 1) * 128],
                                      pt[:].rearrange("p (a b) -> p a b", a=4))
                for n0 in range(0, NX, 512):
                    ps = psS.tile([128, 512], F32, tag="ps")
                    for kt in range(FL // 128):
                        nc.tensor.matmul(ps[:], aT[:, kt, st * 128:(st + 1) * 128],
                                         wproj[:, kt, n0:n0 + 512],
                                         start=(kt == 0), stop=False)
                    nc.tensor.matmul(ps[:], ones_row[:], bp_row[:, n0:n0 + 512],
                                     start=False, stop=True)
                    osb = wp.tile([128, 512], F32, tag="osb")
                    nc.vector.tensor_copy(osb[:], ps[:])
                    if reduce_mode == "rs":
                        nc.sync.dma_start(
                            out=rs_in[st * 128:(st + 1) * 128, n0:n0 + 512],
                            in_=osb[:])
                    else:
                        nc.sync.dma_start(
                            out=out_ext.ap()[st * 128:(st + 1) * 128, n0:n0 + 512],
                            in_=osb[:])

            # ---- emission ----
            for st in range(NK):
                emit_v(st)
            for fb in range(2 * FL // 128):
                emit_qk(fb)
            for qc in range(NQC):
                for lh in range(HL):
                    emit_head_qc(lh, qc)
                for qs in range(4):
                    emit_proj(4 * qc + qs)

            if reduce_mode == "rs":
                nc.gpsimd.collective_compute(
                    "ReduceScatter",
                    mybir.AluOpType.add,
                    replica_groups=[[0, 1], [2, 3], [4, 5], [6, 7]],
                    ins=[rs_in.opt()],
                    outs=[rs_out.opt()],
                )
                nc.sync.dma_start(out=out_ext.ap(), in_=rs_out[:])

    nc.compile()
    return nc


@functools.lru_cache(maxsize=2)
def _built(reduce_mode: str):
    return _build(reduce_mode)


def _in_maps(x, c_attn_w, c_attn_b, c_proj_w, c_proj_b, reduce_mode):
    maps = []
    for core in range(N_CORES):
        b, hg = core // 2, core % 2
        f0 = hg * FL
        w_q = c_attn_w[:, f0:f0 + FL]
        w_k = c_attn_w[:, NX + f0:NX + f0 + FL]
        w_v = c_attn_w[:, 2 * NX + f0:2 * NX + f0 + FL]
        b_q = c_attn_b[f0:f0 + FL]
        b_k = c_attn_b[NX + f0:NX + f0 + FL]
        b_v = c_attn_b[2 * NX + f0:2 * NX + f0 + FL]
        maps.append({
            "xT": np.ascontiguousarray(x[b].T).astype(BF),
            "w_qk": np.concatenate([w_q, w_k], axis=1).astype(BF),
            "w_v": np.ascontiguousarray(w_v).astype(BF),
            "w_proj": np.ascontiguousarray(c_proj_w[f0:f0 + FL, :]).astype(BF),
            "b_qk": np.concatenate([b_q, b_k]).astype(np.float32).reshape(-1, 1),
            "bv_row": b_v.astype(BF).reshape(1, FL),
            "bp_row": (c_proj_b / 2.0).astype(BF).reshape(1, NX),
        })
    return maps


def _run(inputs, reduce_mode=REDUCE_MODE, trace=False):
    nc = _built(reduce_mode)
    maps = _in_maps(inputs["x"], inputs["c_attn_w"], inputs["c_attn_b"],
                    inputs["c_proj_w"], inputs["c_proj_b"], reduce_mode)
    res = run_bass_kernel_spmd(nc, maps, core_ids=list(range(N_CORES)),
                               trace=trace)
    out = np.empty((B, S, NX), dtype=np.float32)
    for b in range(B):
        if reduce_mode == "rs":
            out[b, :S // 2] = res.results[2 * b]["out"]
            out[b, S // 2:] = res.results[2 * b + 1]["out"]
        else:
            out[b] = res.results[2 * b]["out"] + res.results[2 * b + 1]["out"]
    return out, res


def kernel(**inputs):
    out, _ = _run({k: np.asarray(v) for k, v in inputs.items()})
    return out


# revision 10
# speedup vs baseline: 1.2977x; 1.2977x over previous
"""Causal multi-head attention block (B=4, S=2048, NX=1024, H=16, D=64)
distributed over 8 TRN2 NeuronCores.

Sharding: core i handles batch b = i//2 and head-group hg = i%2 (8 of 16
heads).  Each core computes qkv for its heads, causal attention, and a
partial c_proj over its 512 feature rows; the per-batch pair of cores
reduces partials with an on-chip ReduceScatter (or on the host).

All matmuls run in bf16 (f32 PSUM accumulate).  Scores are computed in the
transposed orientation s^T[k, q] = k @ q^T; exp(s^T) tiles then serve as
the stationary operand of u = p @ v_aug (ones-augmented v), which puts the
softmax denominator in a per-partition column -> cheap reciprocal and
per-partition scaling.  A PE-transpose pass rebuilds a^T for c_proj.
"""
import sys

sys.path.insert(0, "/opt/trn_rl_repo")

import functools

import ml_dtypes
import numpy as np

from concourse import bacc, mybir, tile
from concourse.bass_utils import run_bass_kernel_spmd
from concourse.masks import make_identity

B, S, NX = 4, 2048, 1024
H, D = 16, 64
N_CORES = 8
HL = H // 2          # heads per core (local)
FL = HL * D          # local head feature width (512)
BF16 = mybir.dt.bfloat16
F32 = mybir.dt.float32
BF = ml_dtypes.bfloat16

NK = S // 128        # 16 k-tiles of 128
NQC = S // 512       # 4 q-chunks of 512
KK = NX // 128       # 8 contraction blocks

# "rs": on-chip ReduceScatter over core pairs; "host": partials summed on host
REDUCE_MODE = "rs"


def _parse_cfg(cfg: str):
    parts = cfg.split("-")
    d = {"mode": parts[0], "psw": 1536, "psb": 2, "pb": 3}
    for p in parts[1:]:
        if p.startswith("psw"):
            d["psw"] = int(p[3:])
        elif p.startswith("psb"):
            d["psb"] = int(p[3:])
        elif p.startswith("pb"):
            d["pb"] = int(p[2:])
    return d


def _build(cfg: str):
    c = _parse_cfg(cfg)
    reduce_mode = c["mode"]
    PSW, PSB, PB = c["psw"], c["psb"], c["pb"]
    GK = PSW // 512   # full k-tiles per exp group
    nc = bacc.Bacc("TRN2", target_bir_lowering=False, debug=False,
                   num_devices=N_CORES)

    xT_ext = nc.dram_tensor("xT", [NX, S], BF16, kind="ExternalInput")
    wqk_ext = nc.dram_tensor("w_qk", [NX, 2 * FL], BF16, kind="ExternalInput")
    wv_ext = nc.dram_tensor("w_v", [NX, FL], BF16, kind="ExternalInput")
    wp_ext = nc.dram_tensor("w_proj", [FL, NX], BF16, kind="ExternalInput")
    bqk_ext = nc.dram_tensor("b_qk", [2 * FL, 1], F32, kind="ExternalInput")
    bv_ext = nc.dram_tensor("bv_row", [1, FL], BF16, kind="ExternalInput")
    bp_ext = nc.dram_tensor("bp_row", [1, NX], BF16, kind="ExternalInput")
    if reduce_mode == "rs":
        out_ext = nc.dram_tensor("out", [S // 2, NX], F32, kind="ExternalOutput")
    else:
        out_ext = nc.dram_tensor("out", [S, NX], F32, kind="ExternalOutput")

    with tile.TileContext(nc) as tc:
        with tc.tile_pool(name="const", bufs=1) as cp, \
             tc.tile_pool(name="work", bufs=PB) as wp, \
             tc.tile_pool(name="psS", bufs=PSB, space="PSUM") as psS, \
             tc.tile_pool(name="psU", bufs=2, space="PSUM") as psU, \
             tc.tile_pool(name="dram", bufs=1, space="DRAM") as dp:

            # ---- persistent SBUF tensors ----
            xT = cp.tile([128, KK, S], BF16, tag="xT")
            wqk = cp.tile([128, KK, 2 * FL], BF16, tag="wqk")
            wv = cp.tile([128, KK, FL], BF16, tag="wv")
            wproj = cp.tile([128, FL // 128, NX], BF16, tag="wproj")
            qkT = cp.tile([128, 2 * FL // 128, S], BF16, tag="qkT")
            v_aug = cp.tile([128, NK, HL, D + 1], BF16, tag="vaug")
            a_nat = cp.tile([128, NK, FL], BF16, tag="anat")   # a [q, feat]
            aT = cp.tile([128, FL // 128, S], BF16, tag="aT")  # a^T [feat, q]
            bqk = cp.tile([128, 2 * FL // 128], F32, tag="bqk")
            bv_row = cp.tile([1, FL], BF16, tag="bv")
            bp_row = cp.tile([1, NX], BF16, tag="bp")
            ones_row = cp.tile([1, 128], BF16, tag="ones")
            tri = cp.tile([128, 128], BF16, tag="tri")
            ident = cp.tile([128, 128], BF16, tag="ident")

            # ---- input DMAs ----
            for kk in range(KK):
                nc.sync.dma_start(out=xT[:, kk, :],
                                  in_=xT_ext.ap()[kk * 128:(kk + 1) * 128, :])
                nc.sync.dma_start(out=wqk[:, kk, :],
                                  in_=wqk_ext.ap()[kk * 128:(kk + 1) * 128, :])
                nc.sync.dma_start(out=wv[:, kk, :],
                                  in_=wv_ext.ap()[kk * 128:(kk + 1) * 128, :])
            for kt in range(FL // 128):
                nc.sync.dma_start(out=wproj[:, kt, :],
                                  in_=wp_ext.ap()[kt * 128:(kt + 1) * 128, :])
            for fb in range(2 * FL // 128):
                nc.sync.dma_start(out=bqk[:, fb:fb + 1],
                                  in_=bqk_ext.ap()[fb * 128:(fb + 1) * 128, :])
            nc.sync.dma_start(out=bv_row[:], in_=bv_ext.ap())
            nc.sync.dma_start(out=bp_row[:], in_=bp_ext.ap())

            nc.vector.memset(ones_row[:], 1.0)
            # tri[p, f] = 1 if p <= f else 0 (keep-in on p > f, else fill 1)
            nc.vector.memset(tri[:], 0.0)
            nc.gpsimd.affine_select(
                out=tri[:], in_=tri[:],
                compare_op=mybir.AluOpType.is_gt,
                fill=1.0, base=0, pattern=[[-1, 128]], channel_multiplier=1,
            )
            make_identity(nc, ident[:])
            # ones column of v_aug
            nc.vector.memset(v_aug[:, :, :, D:D + 1], 1.0)

            if reduce_mode == "rs":
                rs_in = dp.tile([S, NX], F32)
                rs_out = dp.tile([S // 2, NX], F32)

            # ---- stage 2: v (natural layout) ----
            def emit_v(st):
                ps = psS.tile([128, FL], F32, tag="ps")
                for kk in range(KK):
                    nc.tensor.matmul(ps[:], xT[:, kk, st * 128:(st + 1) * 128],
                                     wv[:, kk, :], start=(kk == 0), stop=False)
                nc.tensor.matmul(ps[:], ones_row[:], bv_row[:],
                                 start=False, stop=True)
                nc.vector.tensor_copy(
                    v_aug[:, st, :, 0:D],
                    ps[:].rearrange("p (h d) -> p h d", d=D))

            # ---- stage 1: q^T / k^T (feature-major) ----
            def emit_qk(fb):
                qk_groups = ((0, 1536), (1536, 512)) if PSW >= 1536 else \
                            ((0, 1024), (1024, 1024))
                for n0, nw in qk_groups:
                    ps = psS.tile([128, nw], F32, tag="ps")
                    for c0 in range(0, nw, 512):
                        for kk in range(KK):
                            nc.tensor.matmul(
                                ps[:, c0:c0 + 512],
                                wqk[:, kk, fb * 128:(fb + 1) * 128],
                                xT[:, kk, n0 + c0:n0 + c0 + 512],
                                start=(kk == 0), stop=(kk == KK - 1))
                    nc.vector.tensor_scalar_add(qkT[:, fb, n0:n0 + nw], ps[:],
                                                bqk[:, fb:fb + 1])

            # ---- stage 3: attention for one (head, q-chunk) ----
            def emit_head_qc(lh, qc):
                fbq = lh // 2
                fbk = FL // 128 + lh // 2
                po = (lh % 2) * 64
                qb = qc * 512
                n_full = 4 * qc
                # groups of k-tiles: full tiles in 3s, then the 4 diagonal
                # tiles packed at PSUM-bank-safe offsets (no matmul output may
                # cross a 512-f32 bank boundary)
                groups = []
                kt0 = 0
                while kt0 < n_full:
                    g = min(GK, n_full - kt0)
                    groups.append([(kt0 + j, j * 512, 512, 0) for j in range(g)])
                    kt0 += g
                if PSW >= 1536:
                    diag_offs = (0, 512, 1024, 1280)
                    groups.append([(n_full + j, diag_offs[j], 512 - 128 * j,
                                    128 * j) for j in range(4)])
                else:
                    groups.append([(n_full + 0, 0, 512, 0),
                                   (n_full + 1, 512, 384, 128)])
                    groups.append([(n_full + 2, 0, 256, 256),
                                   (n_full + 3, 256, 128, 384)])

                pu = psU.tile([128, 4, D + 1], F32, tag="pu")
                last_kt = n_full + 3
                for gi, g in enumerate(groups):
                    is_diag = g[0][0] >= n_full
                    gw = max(off + N for (_, off, N, _) in g)
                    ps = psS.tile([128, PSW], F32, tag="ps")
                    for (kt, off, N, qoff) in g:
                        nc.tensor.matmul(
                            ps[:, off:off + N],
                            qkT[po:po + 64, fbk, kt * 128:(kt + 1) * 128],
                            qkT[po:po + 64, fbq, qb + qoff:qb + 512],
                            start=True, stop=True)
                    p = wp.tile([128, PSW], BF16, tag="p")
                    nc.scalar.activation(p[:, 0:gw], ps[:, 0:gw],
                                         mybir.ActivationFunctionType.Exp,
                                         scale=0.125)
                    if is_diag:
                        for (kt, off, N, qoff) in g:
                            nc.vector.tensor_mul(p[:, off:off + 128],
                                                 p[:, off:off + 128], tri[:])
                    # u accumulation: p slices as stationary, v_aug moving.
                    # start=True clears has_written for the whole PSUM bank,
                    # so it must appear exactly once (first matmul of the
                    # tile); later first-writes of other regions overwrite
                    # because the bank-wide clear reset their has_written.
                    for (kt, off, N, qoff) in g:
                        j = kt - n_full  # >= 0 only for diag tiles
                        for qs in range(qoff // 128, 4):
                            pcol = off + qs * 128 - qoff
                            nc.tensor.matmul(
                                pu[:, qs, :],
                                p[:, pcol:pcol + 128],
                                v_aug[:, kt, lh, :],
                                start=(kt == 0 and qs == 0),
                                stop=(kt == last_kt),
                                skip_group_check=True)
                # normalize into a_nat[q, feat].  The reciprocal reads the
                # sumexp column of ALL 4 q-subtiles in one strided op so it
                # (and the muls that consume it) depend on every u-matmul —
                # a region-scoped read would be hoisted concurrent with PE
                # writes to the same PSUM bank and read torn data.
                recip4 = wp.tile([128, 4], F32, tag="recip")
                nc.vector.reciprocal(
                    recip4[:],
                    pu[:, :, D:D + 1].rearrange("p a b -> p (a b)"))
                for qs in range(4):
                    st = 4 * qc + qs
                    nc.vector.tensor_scalar_mul(
                        a_nat[:, st, lh * D:(lh + 1) * D],
                        pu[:, qs, 0:D], recip4[:, qs:qs + 1])

            # ---- stage 3.5 + 4: transpose a -> a^T, then c_proj partial ----
            def emit_proj(st):
                pt = psU.tile([128, 512], BF16, tag="pu")
                for fb in range(4):
                    nc.tensor.transpose(pt[:, fb * 128:(fb + 1) * 128],
                                        a_nat[:, st, fb * 128:(fb + 1) * 128],
                                        ident[:])
                nc.vector.tensor_copy(aT[:, :, st * 128:(st + 1) * 128],
                                      pt[:].rearrange("p (a b) -> p a b", a=4))
                for n0 in range(0, NX, 512):
                    ps = psS.tile([128, 512], F32, tag="ps")
                    for kt in range(FL // 128):
                        nc.tensor.matmul(ps[:], aT[:, kt, st * 128:(st + 1) * 128],
                                         wproj[:, kt, n0:n0 + 512],
                                         start=(kt == 0), stop=False)
                    nc.tensor.matmul(ps[:], ones_row[:], bp_row[:, n0:n0 + 512],
                                     start=False, stop=True)
                    osb = wp.tile([128, 512], F32, tag="osb")
                    nc.vector.tensor_copy(osb[:], ps[:])
                    if reduce_mode == "rs":
                        nc.sync.dma_start(
                            out=rs_in[st * 128:(st + 1) * 128, n0:n0 + 512],
                            in_=osb[:])
                    else:
                        nc.sync.dma_start(
                            out=out_ext.ap()[st * 128:(st + 1) * 128, n0:n0 + 512],
                            in_=osb[:])

            # ---- emission ----
            for st in range(NK):
                emit_v(st)
            for fb in range(2 * FL // 128):
                emit_qk(fb)
            for qc in range(NQC):
                for lh in range(HL):
                    emit_head_qc(lh, qc)
                for qs in range(4):
                    emit_proj(4 * qc + qs)

            if reduce_mode == "rs":
                nc.gpsimd.collective_compute(
                    "ReduceScatter",
                    mybir.AluOpType.add,
                    replica_groups=[[0, 1], [2, 3], [4, 5], [6, 7]],
                    ins=[rs_in.opt()],
                    outs=[rs_out.opt()],
                )
                nc.sync.dma_start(out=out_ext.ap(), in_=rs_out[:])

    nc.compile()
    return nc


@functools.lru_cache(maxsize=2)
def _built(cfg: str):
    return _build(cfg)


def _in_maps(x, c_attn_w, c_attn_b, c_proj_w, c_proj_b, reduce_mode):
    maps = []
    for core in range(N_CORES):
        b, hg = core // 2, core % 2
        f0 = hg * FL
        w_q = c_attn_w[:, f0:f0 + FL]
        w_k = c_attn_w[:, NX + f0:NX + f0 + FL]
        w_v = c_attn_w[:, 2 * NX + f0:2 * NX + f0 + FL]
        b_q = c_attn_b[f0:f0 + FL]
        b_k = c_attn_b[NX + f0:NX + f0 + FL]
        b_v = c_attn_b[2 * NX + f0:2 * NX + f0 + FL]
        maps.append({
            "xT": np.ascontiguousarray(x[b].T).astype(BF),
            "w_qk": np.concatenate([w_q, w_k], axis=1).astype(BF),
            "w_v": np.ascontiguousarray(w_v).astype(BF),
            "w_proj": np.ascontiguousarray(c_proj_w[f0:f0 + FL, :]).astype(BF),
            "b_qk": np.concatenate([b_q, b_k]).astype(np.float32).reshape(-1, 1),
            "bv_row": b_v.astype(BF).reshape(1, FL),
            "bp_row": (c_proj_b / 2.0).astype(BF).reshape(1, NX),
        })
    return maps


def _run(inputs, cfg=None, trace=False):
    cfg = cfg or REDUCE_MODE
    nc = _built(cfg)
    reduce_mode = _parse_cfg(cfg)["mode"]
    maps = _in_maps(inputs["x"], inputs["c_attn_w"], inputs["c_attn_b"],
                    inputs["c_proj_w"], inputs["c_proj_b"], reduce_mode)
    res = run_bass_kernel_spmd(nc, maps, core_ids=list(range(N_CORES)),
                               trace=trace)
    out = np.empty((B, S, NX), dtype=np.float32)
    for b in range(B):
        if reduce_mode == "rs":
            out[b, :S // 2] = res.results[2 * b]["out"]
            out[b, S // 2:] = res.results[2 * b + 1]["out"]
        else:
            out[b] = res.results[2 * b]["out"] + res.results[2 * b + 1]["out"]
    return out, res


def kernel(**inputs):
    out, _ = _run({k: np.asarray(v) for k, v in inputs.items()})
    return out


# revision 22
# speedup vs baseline: 1.5865x; 1.2226x over previous
"""Causal multi-head attention block (B=4, S=2048, NX=1024, H=16, D=64)
distributed over 8 TRN2 NeuronCores.

Sharding: core i handles batch b = i//2 and head-group hg = i%2 (8 of 16
heads).  Each core computes qkv for its heads, causal attention, and a
partial c_proj over its 512 feature rows; the per-batch pair of cores
reduces partials with an on-chip ReduceScatter (or on the host).

All matmuls run in bf16 (f32 PSUM accumulate).  Scores are computed in the
transposed orientation s^T[k, q] = k @ q^T; exp(s^T) tiles then serve as
the stationary operand of u = p @ v_aug (ones-augmented v), which puts the
softmax denominator in a per-partition column -> cheap reciprocal and
per-partition scaling.  A PE-transpose pass rebuilds a^T for c_proj.
"""
import sys

sys.path.insert(0, "/opt/trn_rl_repo")

import functools

import ml_dtypes
import numpy as np

from concourse import bacc, mybir, tile
from concourse.bass_utils import run_bass_kernel_spmd
from concourse.masks import make_identity

B, S, NX = 4, 2048, 1024
H, D = 16, 64
N_CORES = 8
HL = H // 2          # heads per core (local)
FL = HL * D          # local head feature width (512)
BF16 = mybir.dt.bfloat16
F32 = mybir.dt.float32
BF = ml_dtypes.bfloat16

NK = S // 128        # 16 k-tiles of 128
NQC = S // 512       # 4 q-chunks of 512
KK = NX // 128       # 8 contraction blocks

# Default build configs (see _parse_cfg for the flag grammar).  "host" mode
# returns per-core c_proj partials that the host sums while unsharding; the
# "rs"/"rsc" modes do the reduction on-chip with a pair ReduceScatter.
DEFAULT_CFG = "host-psw1024-psb3-nb-il-sb-pb15-sc"    # biases known zero
DEFAULT_CFG_BIAS = "host-psw1024-psb3-il-sb-pb15-sc"  # general biases


def _parse_cfg(cfg: str):
    parts = cfg.split("-")
    d = {"mode": parts[0], "psw": 1536, "psb": 2, "pb": 3, "nb": False,
         "il": False, "dp": False, "gm": False, "sb": False, "mx": False,
         "ac": False, "do": False, "sc": False}
    for p in parts[1:]:
        if p.startswith("psw"):
            d["psw"] = int(p[3:])
        elif p.startswith("psb"):
            d["psb"] = int(p[3:])
        elif p.startswith("pb"):
            d["pb"] = int(p[2:])
        elif p == "nb":
            d["nb"] = True
        elif p == "il":
            d["il"] = True
        elif p == "dp":
            d["dp"] = True
        elif p == "gm":
            d["gm"] = True
        elif p == "sb":
            d["sb"] = True
        elif p == "mx":
            d["mx"] = True
        elif p == "ac":
            d["ac"] = True
        elif p == "do":
            d["do"] = True
        elif p == "sc":
            d["sc"] = True
    return d


def _build(cfg: str):
    c = _parse_cfg(cfg)
    reduce_mode = c["mode"]
    (PSW, PSB, PB, NB, IL, DP, GM, SBURST, MX, AC, DO, SC) = (
        c["psw"], c["psb"], c["pb"], c["nb"], c["il"], c["dp"], c["gm"],
        c["sb"], c["mx"], c["ac"], c["do"], c["sc"])
    GK = PSW // 512   # full k-tiles per exp group
    nc = bacc.Bacc("TRN2", target_bir_lowering=False, debug=False,
                   num_devices=N_CORES)

    xT_ext = nc.dram_tensor("xT", [NX, S], BF16, kind="ExternalInput")
    wqk_ext = nc.dram_tensor("w_qk", [NX, 2 * FL], BF16, kind="ExternalInput")
    wv_ext = nc.dram_tensor("w_v", [NX, FL], BF16, kind="ExternalInput")
    wp_ext = nc.dram_tensor("w_proj", [FL, NX], BF16, kind="ExternalInput")
    bqk_ext = nc.dram_tensor("b_qk", [2 * FL, 1], F32, kind="ExternalInput")
    bv_ext = nc.dram_tensor("bv_row", [1, FL], BF16, kind="ExternalInput")
    bp_ext = nc.dram_tensor("bp_row", [1, NX], BF16, kind="ExternalInput")
    if reduce_mode in ("rs", "rsc"):
        out_ext = nc.dram_tensor("out", [S // 2, NX], F32, kind="ExternalOutput")
    else:
        out_ext = nc.dram_tensor("out", [S, NX], F32, kind="ExternalOutput")

    with tile.TileContext(nc) as tc:
        with tc.tile_pool(name="const", bufs=1) as cp, \
             tc.tile_pool(name="work", bufs=3) as wp, \
             tc.tile_pool(name="psS", bufs=PSB, space="PSUM") as psS, \
             tc.tile_pool(name="psU", bufs=2, space="PSUM") as psU, \
             tc.tile_pool(name="dram", bufs=1, space="DRAM") as dp:

            # ---- persistent SBUF tensors ----
            xT = cp.tile([128, KK, S], BF16, tag="xT")
            wqk = cp.tile([128, KK, 2 * FL], BF16, tag="wqk")
            wv = cp.tile([128, KK, FL], BF16, tag="wv")
            wproj = cp.tile([128, FL // 128, NX], BF16, tag="wproj")
            qkT = cp.tile([128, 2 * FL // 128, S], BF16, tag="qkT")
            v_aug = cp.tile([128, NK, HL, D + 1], BF16, tag="vaug")
            a_nat = cp.tile([128, NK, FL], BF16, tag="anat")   # a [q, feat]
            aT = cp.tile([128, FL // 128, S], BF16, tag="aT")  # a^T [feat, q]
            bqk = cp.tile([128, 2 * FL // 128], F32, tag="bqk")
            bv_row = cp.tile([1, FL], BF16, tag="bv")
            bp_row = cp.tile([1, NX], BF16, tag="bp")
            ones_row = cp.tile([1, 128], BF16, tag="ones")
            tri = cp.tile([128, 128], BF16, tag="tri")
            ident = cp.tile([128, 128], BF16, tag="ident")

            # ---- input DMAs ----
            for kk in range(KK):
                nc.sync.dma_start(out=xT[:, kk, :],
                                  in_=xT_ext.ap()[kk * 128:(kk + 1) * 128, :])
                nc.sync.dma_start(out=wv[:, kk, :],
                                  in_=wv_ext.ap()[kk * 128:(kk + 1) * 128, :])
            for kk in range(KK):
                nc.sync.dma_start(out=wqk[:, kk, :],
                                  in_=wqk_ext.ap()[kk * 128:(kk + 1) * 128, :])
            for kt in range(FL // 128):
                nc.sync.dma_start(out=wproj[:, kt, :],
                                  in_=wp_ext.ap()[kt * 128:(kt + 1) * 128, :])
            for fb in range(2 * FL // 128):
                nc.sync.dma_start(out=bqk[:, fb:fb + 1],
                                  in_=bqk_ext.ap()[fb * 128:(fb + 1) * 128, :])
            nc.sync.dma_start(out=bv_row[:], in_=bv_ext.ap())
            nc.sync.dma_start(out=bp_row[:], in_=bp_ext.ap())

            nc.vector.memset(ones_row[:], 1.0)
            # tri[p, f] = 1 if p <= f else 0 (keep-in on p > f, else fill 1)
            nc.vector.memset(tri[:], 0.0)
            nc.gpsimd.affine_select(
                out=tri[:], in_=tri[:],
                compare_op=mybir.AluOpType.is_gt,
                fill=1.0, base=0, pattern=[[-1, 128]], channel_multiplier=1,
            )
            make_identity(nc, ident[:])
            gm_zero = nc.gpsimd.to_reg(0.0) if GM else None
            # ones column of v_aug
            nc.vector.memset(v_aug[:, :, :, D:D + 1], 1.0)

            if reduce_mode in ("rs", "rsc"):
                rs_in = dp.tile([S, NX], F32)
                rs_out = dp.tile([S // 2, NX], F32)

            # ---- stage 2: v (natural layout) ----
            def emit_v(st):
                ps = psS.tile([128, FL], F32, tag="ps")
                for kk in range(KK):
                    nc.tensor.matmul(ps[:], xT[:, kk, st * 128:(st + 1) * 128],
                                     wv[:, kk, :], start=(kk == 0),
                                     stop=(NB and kk == KK - 1))
                if not NB:
                    nc.tensor.matmul(ps[:], ones_row[:], bv_row[:],
                                     start=False, stop=True)
                if AC:
                    nc.scalar.copy(v_aug[:, st, :, 0:D],
                                   ps[:].rearrange("p (h d) -> p h d", d=D))
                else:
                    nc.vector.tensor_copy(
                        v_aug[:, st, :, 0:D],
                        ps[:].rearrange("p (h d) -> p h d", d=D))

            # ---- stage 1: q^T / k^T (feature-major) ----
            def emit_qk(fb):
                qk_groups = ((0, 1536), (1536, 512)) if PSW >= 1536 else \
                            ((0, 1024), (1024, 1024))
                for n0, nw in qk_groups:
                    ps = psS.tile([128, nw], F32, tag="ps")
                    for c0 in range(0, nw, 512):
                        for kk in range(KK):
                            nc.tensor.matmul(
                                ps[:, c0:c0 + 512],
                                wqk[:, kk, fb * 128:(fb + 1) * 128],
                                xT[:, kk, n0 + c0:n0 + c0 + 512],
                                start=(kk == 0), stop=(kk == KK - 1))
                    if AC:
                        nc.scalar.activation(
                            qkT[:, fb, n0:n0 + nw], ps[:],
                            mybir.ActivationFunctionType.Identity,
                            bias=bqk[:, fb:fb + 1])
                    elif SC:
                        for s0 in range(0, nw, 512):
                            nc.vector.tensor_scalar_add(
                                qkT[:, fb, n0 + s0:n0 + s0 + 512],
                                ps[:, s0:s0 + 512], bqk[:, fb:fb + 1])
                    else:
                        nc.vector.tensor_scalar_add(qkT[:, fb, n0:n0 + nw],
                                                    ps[:], bqk[:, fb:fb + 1])

            # ---- stage 3: attention, software-pipelined over head pairs ----
            def head_groups(lh, qc):
                """Return (emit_scores, emit_u) closures per k-tile group plus
                a finalize closure, sharing pu/p tile state."""
                fbq = lh // 2
                fbk = FL // 128 + lh // 2
                po = (lh % 2) * 64
                qb = qc * 512
                n_full = 4 * qc
                groups = []
                kt0 = 0
                while kt0 < n_full:
                    g = min(GK, n_full - kt0)
                    groups.append([(kt0 + j, j * 512, 512, 0) for j in range(g)])
                    kt0 += g
                if PSW >= 1536:
                    diag_offs = (0, 512, 1024, 1280)
                    groups.append([(n_full + j, diag_offs[j], 512 - 128 * j,
                                    128 * j) for j in range(4)])
                else:
                    groups.append([(n_full + 0, 0, 512, 0),
                                   (n_full + 1, 512, 384, 128)])
                    groups.append([(n_full + 2, 0, 256, 256),
                                   (n_full + 3, 256, 128, 384)])

                state = {"pu": None}
                p_tiles = [None] * len(groups)
                last_kt = n_full + 3

                def mk_scores(gi, g):
                    def emit():
                        gw = max(off + N for (_, off, N, _) in g)
                        ps = psS.tile([128, PSW], F32, tag="ps")
                        for (kt, off, N, qoff) in g:
                            nc.tensor.matmul(
                                ps[:, off:off + N],
                                qkT[po:po + 64, fbk, kt * 128:(kt + 1) * 128],
                                qkT[po:po + 64, fbq, qb + qoff:qb + 512],
                                start=True, stop=True)
                        p = wp.tile([128, PSW], BF16, tag="p", bufs=PB)
                        nc.scalar.activation(p[:, 0:gw], ps[:, 0:gw],
                                             mybir.ActivationFunctionType.Exp,
                                             scale=0.125)
                        if g[0][0] >= n_full:
                            for (kt, off, N, qoff) in g:
                                if GM:
                                    nc.gpsimd.affine_select(
                                        out=p[:, off:off + 128],
                                        in_=p[:, off:off + 128],
                                        compare_op=mybir.AluOpType.is_le,
                                        fill=gm_zero, base=0,
                                        pattern=[[-1, 128]],
                                        channel_multiplier=1)
                                else:
                                    nc.vector.tensor_mul(p[:, off:off + 128],
                                                         p[:, off:off + 128],
                                                         tri[:])
                        p_tiles[gi] = p
                    return emit

                def mk_u(gi, g):
                    def emit():
                        if state["pu"] is None:
                            state["pu"] = psU.tile([128, 4, D + 1], F32, tag="pu", name="pu_t")
                        pu = state["pu"]
                        p = p_tiles[gi]
                        for (kt, off, N, qoff) in g:
                            for qs in range(qoff // 128, 4):
                                pcol = off + qs * 128 - qoff
                                nc.tensor.matmul(
                                    pu[:, qs, :],
                                    p[:, pcol:pcol + 128],
                                    v_aug[:, kt, lh, :],
                                    start=(kt == 0 and qs == 0),
                                    stop=(kt == last_kt),
                                    skip_group_check=True)
                    return emit

                def finalize():
                    pu = state["pu"]
                    recip4 = wp.tile([128, 4], F32, tag="recip")
                    nc.vector.reciprocal(
                        recip4[:],
                        pu[:, :, D:D + 1].rearrange("p a b -> p (a b)"))
                    for qs in range(4):
                        st = 4 * qc + qs
                        nc.vector.tensor_scalar_mul(
                            a_nat[:, st, lh * D:(lh + 1) * D],
                            pu[:, qs, 0:D], recip4[:, qs:qs + 1])

                return ([(mk_scores(gi, g), mk_u(gi, g))
                         for gi, g in enumerate(groups)], finalize)

            def emit_pair(lhA, lhB, qc):
                SA, finA = head_groups(lhA, qc)
                SB, finB = head_groups(lhB, qc)
                n = len(SA)
                if SBURST:
                    for i in range(n):
                        SA[i][0]()
                        SB[i][0]()
                    for i in range(n):
                        SA[i][1]()
                        SB[i][1]()
                    finA()
                    finB()
                    return
                if not DP:
                    SA[0][0]()
                    for i in range(n):
                        SB[i][0]()
                        SA[i][1]()
                        if i + 1 < n:
                            SA[i + 1][0]()
                        SB[i][1]()
                    finA()
                    finB()
                    return
                # look-ahead-2: each head's u lags its scores by 2 groups
                units = []
                for i in range(n):
                    units.append(SA[i][0])
                    units.append(SB[i][0])
                us = []
                for i in range(n):
                    us.append(SA[i][1])
                    us.append(SB[i][1])
                out = []
                si = 0
                for k in range(len(units) + len(us)):
                    # emit scores until 3 ahead of u count, then alternate
                    pass
                si, ui = 0, 0
                while si < len(units) or ui < len(us):
                    if si < len(units) and si < ui + 3:
                        units[si]()
                        si += 1
                    else:
                        us[ui]()
                        ui += 1
                finA()
                finB()

            # ---- stage 3.5 + 4: transpose a -> a^T, then c_proj partial ----
            def emit_proj(st):
                pt = psU.tile([128, 512], BF16, tag="pu")
                for fb in range(4):
                    nc.tensor.transpose(pt[:, fb * 128:(fb + 1) * 128],
                                        a_nat[:, st, fb * 128:(fb + 1) * 128],
                                        ident[:])
                if AC:
                    nc.scalar.copy(aT[:, :, st * 128:(st + 1) * 128],
                                   pt[:].rearrange("p (a b) -> p a b", a=4))
                else:
                    nc.vector.tensor_copy(aT[:, :, st * 128:(st + 1) * 128],
                                          pt[:].rearrange("p (a b) -> p a b",
                                                          a=4))
                for n0 in range(0, NX, 512):
                    ps = psS.tile([128, 512], F32, tag="ps")
                    for kt in range(FL // 128):
                        nc.tensor.matmul(ps[:], aT[:, kt, st * 128:(st + 1) * 128],
                                         wproj[:, kt, n0:n0 + 512],
                                         start=(kt == 0),
                                         stop=(NB and kt == FL // 128 - 1))
                    if not NB:
                        nc.tensor.matmul(ps[:], ones_row[:],
                                         bp_row[:, n0:n0 + 512],
                                         start=False, stop=True)
                    if reduce_mode in ("rs", "rsc"):
                        dst = rs_in[st * 128:(st + 1) * 128, n0:n0 + 512]
                    else:
                        dst = out_ext.ap()[st * 128:(st + 1) * 128, n0:n0 + 512]
                    if DO:
                        nc.sync.dma_start(out=dst, in_=ps[:])
                    else:
                        osb = wp.tile([128, 512], F32, tag="osb")
                        if AC:
                            nc.scalar.copy(osb[:], ps[:])
                        else:
                            nc.vector.tensor_copy(osb[:], ps[:])
                        nc.sync.dma_start(out=dst, in_=osb[:])

            def emit_rs_chunk(qc):
                # reduce-scatter one q-chunk [512, NX] across the core pair;
                # each core keeps [256, NX] at row offset qc*256
                r0 = qc * 512
                nc.gpsimd.collective_compute(
                    "ReduceScatter",
                    mybir.AluOpType.add,
                    replica_groups=[[0, 1], [2, 3], [4, 5], [6, 7]],
                    ins=[rs_in[r0:r0 + 512, :].opt()],
                    outs=[rs_out[qc * 256:(qc + 1) * 256, :].opt()],
                )
                nc.sync.dma_start(
                    out=out_ext.ap()[qc * 256:(qc + 1) * 256, :],
                    in_=rs_out[qc * 256:(qc + 1) * 256, :])

            # ---- emission ----
            if MX:
                # spread dense (warm) qk/v/proj work between attention pairs
                for st in range(4):
                    emit_v(st)
                emit_qk(0)
                emit_qk(FL // 128)
                emit_pair(0, 1, 0)
                for pr in range(1, HL // 2):
                    emit_qk(pr)
                    emit_qk(FL // 128 + pr)
                    emit_pair(2 * pr, 2 * pr + 1, 0)
                fill = {
                    (1, 0): [("v", 4), ("v", 5), ("v", 6), ("v", 7)],
                    (1, 1): [("v", 8), ("v", 9), ("v", 10), ("v", 11)],
                    (1, 2): [("v", 12), ("v", 13), ("v", 14), ("v", 15)],
                    (1, 3): [("p", 0)],
                    (2, 0): [("p", 1)], (2, 1): [("p", 2)],
                    (2, 2): [("p", 3)], (2, 3): [("p", 4)],
                    (3, 0): [("p", 5), ("p", 6)], (3, 1): [("p", 7), ("p", 8)],
                    (3, 2): [("p", 9), ("p", 10)],
                    (3, 3): [("p", 11), ("p", 12)],
                }
                for qc in range(1, NQC):
                    for pr in range(HL // 2):
                        for kind, idx in fill.get((qc, pr), []):
                            if kind == "v":
                                emit_v(idx)
                            else:
                                emit_proj(idx)
                        emit_pair(2 * pr, 2 * pr + 1, qc)
                for st in range(13, NK):
                    emit_proj(st)
                if reduce_mode == "rsc":
                    for qc in range(NQC):
                        emit_rs_chunk(qc)
            else:
                if IL:
                    for st in range(4):
                        emit_v(st)
                    emit_qk(0)
                    emit_qk(FL // 128)
                    emit_pair(0, 1, 0)
                    for st in range(4, NK):
                        emit_v(st)
                    for pr in range(1, HL // 2):
                        emit_qk(pr)
                        emit_qk(FL // 128 + pr)
                        emit_pair(2 * pr, 2 * pr + 1, 0)
                else:
                    for st in range(NK):
                        emit_v(st)
                    for fb in range(2 * FL // 128):
                        emit_qk(fb)
                    for pr in range(HL // 2):
                        emit_pair(2 * pr, 2 * pr + 1, 0)
                for qc in range(1, NQC):
                    for pr in range(HL // 2):
                        emit_pair(2 * pr, 2 * pr + 1, qc)
                    for qs in range(4):
                        emit_proj(4 * (qc - 1) + qs)
                    if reduce_mode == "rsc":
                        emit_rs_chunk(qc - 1)
                for qs in range(4):
                    emit_proj(4 * (NQC - 1) + qs)
                if reduce_mode == "rsc":
                    emit_rs_chunk(NQC - 1)

            if reduce_mode == "rs":
                nc.gpsimd.collective_compute(
                    "ReduceScatter",
                    mybir.AluOpType.add,
                    replica_groups=[[0, 1], [2, 3], [4, 5], [6, 7]],
                    ins=[rs_in.opt()],
                    outs=[rs_out.opt()],
                )
                nc.sync.dma_start(out=out_ext.ap(), in_=rs_out[:])

    nc.compile()
    return nc


@functools.lru_cache(maxsize=2)
def _built(cfg: str):
    return _build(cfg)


def _in_maps(x, c_attn_w, c_attn_b, c_proj_w, c_proj_b, reduce_mode):
    maps = []
    for core in range(N_CORES):
        b, hg = core // 2, core % 2
        f0 = hg * FL
        w_q = c_attn_w[:, f0:f0 + FL]
        w_k = c_attn_w[:, NX + f0:NX + f0 + FL]
        w_v = c_attn_w[:, 2 * NX + f0:2 * NX + f0 + FL]
        b_q = c_attn_b[f0:f0 + FL]
        b_k = c_attn_b[NX + f0:NX + f0 + FL]
        b_v = c_attn_b[2 * NX + f0:2 * NX + f0 + FL]
        maps.append({
            "xT": np.ascontiguousarray(x[b].T).astype(BF),
            "w_qk": np.concatenate([w_q, w_k], axis=1).astype(BF),
            "w_v": np.ascontiguousarray(w_v).astype(BF),
            "w_proj": np.ascontiguousarray(c_proj_w[f0:f0 + FL, :]).astype(BF),
            "b_qk": np.concatenate([b_q, b_k]).astype(np.float32).reshape(-1, 1),
            "bv_row": b_v.astype(BF).reshape(1, FL),
            "bp_row": (c_proj_b / 2.0).astype(BF).reshape(1, NX),
        })
    return maps


def _run(inputs, cfg=None, trace=False):
    if cfg is None:
        zero_bias = (not inputs["c_attn_b"].any()) and \
                    (not inputs["c_proj_b"].any())
        cfg = DEFAULT_CFG if zero_bias else DEFAULT_CFG_BIAS
    nc = _built(cfg)
    reduce_mode = _parse_cfg(cfg)["mode"]
    maps = _in_maps(inputs["x"], inputs["c_attn_w"], inputs["c_attn_b"],
                    inputs["c_proj_w"], inputs["c_proj_b"], reduce_mode)
    res = run_bass_kernel_spmd(nc, maps, core_ids=list(range(N_CORES)),
                               trace=trace)
    out = np.empty((B, S, NX), dtype=np.float32)
    for b in range(B):
        if reduce_mode == "rs":
            out[b, :S // 2] = res.results[2 * b]["out"]
            out[b, S // 2:] = res.results[2 * b + 1]["out"]
        elif reduce_mode == "rsc":
            o0 = res.results[2 * b]["out"].reshape(4, 256, NX)
            o1 = res.results[2 * b + 1]["out"].reshape(4, 256, NX)
            out[b] = np.concatenate([o0, o1], axis=1).reshape(S, NX)
        else:
            out[b] = res.results[2 * b]["out"] + res.results[2 * b + 1]["out"]
    return out, res


def kernel(**inputs):
    out, _ = _run({k: np.asarray(v) for k, v in inputs.items()})
    return out


# revision 24
# speedup vs baseline: 1.6153x; 1.0182x over previous
"""Causal multi-head attention block (B=4, S=2048, NX=1024, H=16, D=64)
distributed over 8 TRN2 NeuronCores.

Sharding: core i handles batch b = i//2 and head-group hg = i%2 (8 of 16
heads).  Each core computes qkv for its heads, causal attention, and a
partial c_proj over its 512 feature rows; the per-batch pair of cores
reduces partials with an on-chip ReduceScatter (or on the host).

All matmuls run in bf16 (f32 PSUM accumulate).  Scores are computed in the
transposed orientation s^T[k, q] = k @ q^T; exp(s^T) tiles then serve as
the stationary operand of u = p @ v_aug (ones-augmented v), which puts the
softmax denominator in a per-partition column -> cheap reciprocal and
per-partition scaling.  A PE-transpose pass rebuilds a^T for c_proj.
"""
import sys

sys.path.insert(0, "/opt/trn_rl_repo")

import functools

import ml_dtypes
import numpy as np

from concourse import bacc, mybir, tile
from concourse.bass_utils import run_bass_kernel_spmd
from concourse.masks import make_identity

B, S, NX = 4, 2048, 1024
H, D = 16, 64
N_CORES = 8
HL = H // 2          # heads per core (local)
FL = HL * D          # local head feature width (512)
BF16 = mybir.dt.bfloat16
F32 = mybir.dt.float32
BF = ml_dtypes.bfloat16

NK = S // 128        # 16 k-tiles of 128
NQC = S // 512       # 4 q-chunks of 512
KK = NX // 128       # 8 contraction blocks

# Default build configs (see _parse_cfg for the flag grammar).  "host" mode
# returns per-core c_proj partials that the host sums while unsharding; the
# "rs"/"rsc" modes do the reduction on-chip with a pair ReduceScatter.
DEFAULT_CFG = "host-psw1024-psb3-nb-il-sb-pb15-sc"    # biases known zero
DEFAULT_CFG_BIAS = "host-psw1024-psb3-il-sb-pb15-sc"  # general biases


def _parse_cfg(cfg: str):
    parts = cfg.split("-")
    d = {"mode": parts[0], "psw": 1536, "psb": 2, "pb": 3, "nb": False,
         "il": False, "dp": False, "gm": False, "sb": False, "mx": False,
         "ac": False, "do": False, "sc": False, "dt": False, "pub": 2}
    for p in parts[1:]:
        if p.startswith("psw"):
            d["psw"] = int(p[3:])
        elif p.startswith("psb"):
            d["psb"] = int(p[3:])
        elif p.startswith("pb"):
            d["pb"] = int(p[2:])
        elif p == "nb":
            d["nb"] = True
        elif p == "il":
            d["il"] = True
        elif p == "dp":
            d["dp"] = True
        elif p == "gm":
            d["gm"] = True
        elif p == "sb":
            d["sb"] = True
        elif p == "mx":
            d["mx"] = True
        elif p == "ac":
            d["ac"] = True
        elif p == "do":
            d["do"] = True
        elif p == "sc":
            d["sc"] = True
        elif p == "dt":
            d["dt"] = True
        elif p.startswith("pub"):
            d["pub"] = int(p[3:])
    return d


def _build(cfg: str):
    c = _parse_cfg(cfg)
    reduce_mode = c["mode"]
    (PSW, PSB, PB, NB, IL, DP, GM, SBURST, MX, AC, DO, SC, DT, PUB) = (
        c["psw"], c["psb"], c["pb"], c["nb"], c["il"], c["dp"], c["gm"],
        c["sb"], c["mx"], c["ac"], c["do"], c["sc"], c["dt"], c["pub"])
    GK = PSW // 512   # full k-tiles per exp group
    nc = bacc.Bacc("TRN2", target_bir_lowering=False, debug=False,
                   num_devices=N_CORES)

    xT_ext = nc.dram_tensor("xT", [NX, S], BF16, kind="ExternalInput")
    wqk_ext = nc.dram_tensor("w_qk", [NX, 2 * FL], BF16, kind="ExternalInput")
    wv_ext = nc.dram_tensor("w_v", [NX, FL], BF16, kind="ExternalInput")
    wp_ext = nc.dram_tensor("w_proj", [FL, NX], BF16, kind="ExternalInput")
    bqk_ext = nc.dram_tensor("b_qk", [2 * FL, 1], F32, kind="ExternalInput")
    bv_ext = nc.dram_tensor("bv_row", [1, FL], BF16, kind="ExternalInput")
    bp_ext = nc.dram_tensor("bp_row", [1, NX], BF16, kind="ExternalInput")
    if reduce_mode in ("rs", "rsc"):
        out_ext = nc.dram_tensor("out", [S // 2, NX], F32, kind="ExternalOutput")
    else:
        out_ext = nc.dram_tensor("out", [S, NX], F32, kind="ExternalOutput")

    with tile.TileContext(nc) as tc:
        with tc.tile_pool(name="const", bufs=1) as cp, \
             tc.tile_pool(name="work", bufs=3) as wp, \
             tc.tile_pool(name="psS", bufs=PSB, space="PSUM") as psS, \
             tc.tile_pool(name="psU", bufs=PUB, space="PSUM") as psU, \
             tc.tile_pool(name="dram", bufs=1, space="DRAM") as dp:

            # ---- persistent SBUF tensors ----
            xT = cp.tile([128, KK, S], BF16, tag="xT")
            wqk = cp.tile([128, KK, 2 * FL], BF16, tag="wqk")
            wv = cp.tile([128, KK, FL], BF16, tag="wv")
            wproj = cp.tile([128, FL // 128, NX], BF16, tag="wproj")
            qkT = cp.tile([128, 2 * FL // 128, S], BF16, tag="qkT")
            v_aug = cp.tile([128, NK, HL, D + 1], BF16, tag="vaug")
            a_nat = cp.tile([128, NK, FL], BF16, tag="anat")   # a [q, feat]
            aT = cp.tile([128, FL // 128, S], BF16, tag="aT")  # a^T [feat, q]
            bqk = cp.tile([128, 2 * FL // 128], F32, tag="bqk")
            bv_row = cp.tile([1, FL], BF16, tag="bv")
            bp_row = cp.tile([1, NX], BF16, tag="bp")
            ones_row = cp.tile([1, 128], BF16, tag="ones")
            tri = cp.tile([128, 128], BF16, tag="tri")
            ident = cp.tile([128, 128], BF16, tag="ident")

            # ---- input DMAs ----
            for kk in range(KK):
                nc.sync.dma_start(out=xT[:, kk, :],
                                  in_=xT_ext.ap()[kk * 128:(kk + 1) * 128, :])
                nc.sync.dma_start(out=wv[:, kk, :],
                                  in_=wv_ext.ap()[kk * 128:(kk + 1) * 128, :])
            for kk in range(KK):
                nc.sync.dma_start(out=wqk[:, kk, :],
                                  in_=wqk_ext.ap()[kk * 128:(kk + 1) * 128, :])
            for kt in range(FL // 128):
                nc.sync.dma_start(out=wproj[:, kt, :],
                                  in_=wp_ext.ap()[kt * 128:(kt + 1) * 128, :])
            for fb in range(2 * FL // 128):
                nc.sync.dma_start(out=bqk[:, fb:fb + 1],
                                  in_=bqk_ext.ap()[fb * 128:(fb + 1) * 128, :])
            nc.sync.dma_start(out=bv_row[:], in_=bv_ext.ap())
            nc.sync.dma_start(out=bp_row[:], in_=bp_ext.ap())

            nc.vector.memset(ones_row[:], 1.0)
            # tri[p, f] = 1 if p <= f else 0 (keep-in on p > f, else fill 1)
            nc.vector.memset(tri[:], 0.0)
            nc.gpsimd.affine_select(
                out=tri[:], in_=tri[:],
                compare_op=mybir.AluOpType.is_gt,
                fill=1.0, base=0, pattern=[[-1, 128]], channel_multiplier=1,
            )
            make_identity(nc, ident[:])
            gm_zero = nc.gpsimd.to_reg(0.0) if GM else None
            # ones column of v_aug
            nc.vector.memset(v_aug[:, :, :, D:D + 1], 1.0)

            if reduce_mode in ("rs", "rsc"):
                rs_in = dp.tile([S, NX], F32)
                rs_out = dp.tile([S // 2, NX], F32)

            # ---- stage 2: v (natural layout) ----
            def emit_v(st):
                ps = psS.tile([128, FL], F32, tag="ps")
                for kk in range(KK):
                    nc.tensor.matmul(ps[:], xT[:, kk, st * 128:(st + 1) * 128],
                                     wv[:, kk, :], start=(kk == 0),
                                     stop=(NB and kk == KK - 1))
                if not NB:
                    nc.tensor.matmul(ps[:], ones_row[:], bv_row[:],
                                     start=False, stop=True)
                if AC:
                    nc.scalar.copy(v_aug[:, st, :, 0:D],
                                   ps[:].rearrange("p (h d) -> p h d", d=D))
                else:
                    nc.vector.tensor_copy(
                        v_aug[:, st, :, 0:D],
                        ps[:].rearrange("p (h d) -> p h d", d=D))

            # ---- stage 1: q^T / k^T (feature-major) ----
            def emit_qk(fb):
                qk_groups = ((0, 1536), (1536, 512)) if PSW >= 1536 else \
                            ((0, 1024), (1024, 1024))
                for n0, nw in qk_groups:
                    ps = psS.tile([128, nw], F32, tag="ps")
                    for c0 in range(0, nw, 512):
                        for kk in range(KK):
                            nc.tensor.matmul(
                                ps[:, c0:c0 + 512],
                                wqk[:, kk, fb * 128:(fb + 1) * 128],
                                xT[:, kk, n0 + c0:n0 + c0 + 512],
                                start=(kk == 0), stop=(kk == KK - 1))
                    if AC:
                        nc.scalar.activation(
                            qkT[:, fb, n0:n0 + nw], ps[:],
                            mybir.ActivationFunctionType.Identity,
                            bias=bqk[:, fb:fb + 1])
                    elif SC:
                        for s0 in range(0, nw, 512):
                            nc.vector.tensor_scalar_add(
                                qkT[:, fb, n0 + s0:n0 + s0 + 512],
                                ps[:, s0:s0 + 512], bqk[:, fb:fb + 1])
                    else:
                        nc.vector.tensor_scalar_add(qkT[:, fb, n0:n0 + nw],
                                                    ps[:], bqk[:, fb:fb + 1])

            # ---- stage 3: attention, software-pipelined over head pairs ----
            def head_groups(lh, qc):
                """Return (emit_scores, emit_u) closures per k-tile group plus
                a finalize closure, sharing pu/p tile state."""
                fbq = lh // 2
                fbk = FL // 128 + lh // 2
                po = (lh % 2) * 64
                qb = qc * 512
                n_full = 4 * qc
                groups = []
                kt0 = 0
                while kt0 < n_full:
                    g = min(GK, n_full - kt0)
                    groups.append([(kt0 + j, j * 512, 512, 0) for j in range(g)])
                    kt0 += g
                if PSW >= 1536:
                    diag_offs = (0, 512, 1024, 1280)
                    groups.append([(n_full + j, diag_offs[j], 512 - 128 * j,
                                    128 * j) for j in range(4)])
                else:
                    groups.append([(n_full + 0, 0, 512, 0),
                                   (n_full + 1, 512, 384, 128)])
                    groups.append([(n_full + 2, 0, 256, 256),
                                   (n_full + 3, 256, 128, 384)])

                state = {"pu": None}
                p_tiles = [None] * len(groups)
                last_kt = n_full + 3

                def mk_scores(gi, g):
                    def emit():
                        gw = max(off + N for (_, off, N, _) in g)
                        ps = psS.tile([128, PSW], F32, tag="ps")
                        for (kt, off, N, qoff) in g:
                            nc.tensor.matmul(
                                ps[:, off:off + N],
                                qkT[po:po + 64, fbk, kt * 128:(kt + 1) * 128],
                                qkT[po:po + 64, fbq, qb + qoff:qb + 512],
                                start=True, stop=True)
                        p = wp.tile([128, PSW], BF16, tag="p", bufs=PB)
                        nc.scalar.activation(p[:, 0:gw], ps[:, 0:gw],
                                             mybir.ActivationFunctionType.Exp,
                                             scale=0.125)
                        if g[0][0] >= n_full:
                            for (kt, off, N, qoff) in g:
                                if GM:
                                    nc.gpsimd.affine_select(
                                        out=p[:, off:off + 128],
                                        in_=p[:, off:off + 128],
                                        compare_op=mybir.AluOpType.is_le,
                                        fill=gm_zero, base=0,
                                        pattern=[[-1, 128]],
                                        channel_multiplier=1)
                                else:
                                    nc.vector.tensor_mul(p[:, off:off + 128],
                                                         p[:, off:off + 128],
                                                         tri[:])
                        p_tiles[gi] = p
                    return emit

                def mk_u(gi, g):
                    def emit():
                        if state["pu"] is None:
                            state["pu"] = psU.tile([128, 4, D + 1], F32, tag="pu", name="pu_t")
                        pu = state["pu"]
                        p = p_tiles[gi]
                        for (kt, off, N, qoff) in g:
                            for qs in range(qoff // 128, 4):
                                pcol = off + qs * 128 - qoff
                                nc.tensor.matmul(
                                    pu[:, qs, :],
                                    p[:, pcol:pcol + 128],
                                    v_aug[:, kt, lh, :],
                                    start=(kt == 0 and qs == 0),
                                    stop=(kt == last_kt),
                                    skip_group_check=True)
                    return emit

                def finalize():
                    pu = state["pu"]
                    recip4 = wp.tile([128, 4], F32, tag="recip")
                    nc.vector.reciprocal(
                        recip4[:],
                        pu[:, :, D:D + 1].rearrange("p a b -> p (a b)"))
                    for qs in range(4):
                        st = 4 * qc + qs
                        nc.vector.tensor_scalar_mul(
                            a_nat[:, st, lh * D:(lh + 1) * D],
                            pu[:, qs, 0:D], recip4[:, qs:qs + 1])

                return ([(mk_scores(gi, g), mk_u(gi, g))
                         for gi, g in enumerate(groups)], finalize)

            def emit_pair(lhA, lhB, qc):
                SA, finA = head_groups(lhA, qc)
                SB, finB = head_groups(lhB, qc)
                n = len(SA)
                if SBURST:
                    for i in range(n):
                        SA[i][0]()
                        SB[i][0]()
                    for i in range(n):
                        SA[i][1]()
                        SB[i][1]()
                    finA()
                    finB()
                    return
                if not DP:
                    SA[0][0]()
                    for i in range(n):
                        SB[i][0]()
                        SA[i][1]()
                        if i + 1 < n:
                            SA[i + 1][0]()
                        SB[i][1]()
                    finA()
                    finB()
                    return
                # look-ahead-2: each head's u lags its scores by 2 groups
                units = []
                for i in range(n):
                    units.append(SA[i][0])
                    units.append(SB[i][0])
                us = []
                for i in range(n):
                    us.append(SA[i][1])
                    us.append(SB[i][1])
                out = []
                si = 0
                for k in range(len(units) + len(us)):
                    # emit scores until 3 ahead of u count, then alternate
                    pass
                si, ui = 0, 0
                while si < len(units) or ui < len(us):
                    if si < len(units) and si < ui + 3:
                        units[si]()
                        si += 1
                    else:
                        us[ui]()
                        ui += 1
                finA()
                finB()

            # ---- stage 3.5 + 4: transpose a -> a^T, then c_proj partial ----
            def emit_proj(st):
                if DT:
                    nc.sync.dma_start_transpose(
                        out=aT[:, :, st * 128:(st + 1) * 128],
                        in_=a_nat[:, st, :])
                else:
                    pt = psU.tile([128, 512], BF16, tag="pu")
                    for fb in range(4):
                        nc.tensor.transpose(pt[:, fb * 128:(fb + 1) * 128],
                                            a_nat[:, st, fb * 128:(fb + 1) * 128],
                                            ident[:])
                    if AC:
                        nc.scalar.copy(aT[:, :, st * 128:(st + 1) * 128],
                                       pt[:].rearrange("p (a b) -> p a b", a=4))
                    else:
                        nc.vector.tensor_copy(aT[:, :, st * 128:(st + 1) * 128],
                                              pt[:].rearrange("p (a b) -> p a b",
                                                              a=4))
                for n0 in range(0, NX, 512):
                    ps = psS.tile([128, 512], F32, tag="ps")
                    for kt in range(FL // 128):
                        nc.tensor.matmul(ps[:], aT[:, kt, st * 128:(st + 1) * 128],
                                         wproj[:, kt, n0:n0 + 512],
                                         start=(kt == 0),
                                         stop=(NB and kt == FL // 128 - 1))
                    if not NB:
                        nc.tensor.matmul(ps[:], ones_row[:],
                                         bp_row[:, n0:n0 + 512],
                                         start=False, stop=True)
                    if reduce_mode in ("rs", "rsc"):
                        dst = rs_in[st * 128:(st + 1) * 128, n0:n0 + 512]
                    else:
                        dst = out_ext.ap()[st * 128:(st + 1) * 128, n0:n0 + 512]
                    if DO:
                        nc.sync.dma_start(out=dst, in_=ps[:])
                    else:
                        osb = wp.tile([128, 512], F32, tag="osb")
                        if AC:
                            nc.scalar.copy(osb[:], ps[:])
                        else:
                            nc.vector.tensor_copy(osb[:], ps[:])
                        nc.sync.dma_start(out=dst, in_=osb[:])

            def emit_rs_chunk(qc):
                # reduce-scatter one q-chunk [512, NX] across the core pair;
                # each core keeps [256, NX] at row offset qc*256
                r0 = qc * 512
                nc.gpsimd.collective_compute(
                    "ReduceScatter",
                    mybir.AluOpType.add,
                    replica_groups=[[0, 1], [2, 3], [4, 5], [6, 7]],
                    ins=[rs_in[r0:r0 + 512, :].opt()],
                    outs=[rs_out[qc * 256:(qc + 1) * 256, :].opt()],
                )
                nc.sync.dma_start(
                    out=out_ext.ap()[qc * 256:(qc + 1) * 256, :],
                    in_=rs_out[qc * 256:(qc + 1) * 256, :])

            # ---- emission ----
            if MX:
                # spread dense (warm) qk/v/proj work between attention pairs
                for st in range(4):
                    emit_v(st)
                emit_qk(0)
                emit_qk(FL // 128)
                emit_pair(0, 1, 0)
                for pr in range(1, HL // 2):
                    emit_qk(pr)
                    emit_qk(FL // 128 + pr)
                    emit_pair(2 * pr, 2 * pr + 1, 0)
                fill = {
                    (1, 0): [("v", 4), ("v", 5), ("v", 6), ("v", 7)],
                    (1, 1): [("v", 8), ("v", 9), ("v", 10), ("v", 11)],
                    (1, 2): [("v", 12), ("v", 13), ("v", 14), ("v", 15)],
                    (1, 3): [("p", 0)],
                    (2, 0): [("p", 1)], (2, 1): [("p", 2)],
                    (2, 2): [("p", 3)], (2, 3): [("p", 4)],
                    (3, 0): [("p", 5), ("p", 6)], (3, 1): [("p", 7), ("p", 8)],
                    (3, 2): [("p", 9), ("p", 10)],
                    (3, 3): [("p", 11), ("p", 12)],
                }
                for qc in range(1, NQC):
                    for pr in range(HL // 2):
                        for kind, idx in fill.get((qc, pr), []):
                            if kind == "v":
                                emit_v(idx)
                            else:
                                emit_proj(idx)
                        emit_pair(2 * pr, 2 * pr + 1, qc)
                for st in range(13, NK):
                    emit_proj(st)
                if reduce_mode == "rsc":
                    for qc in range(NQC):
                        emit_rs_chunk(qc)
            else:
                if IL:
                    for st in range(4):
                        emit_v(st)
                    emit_qk(0)
                    emit_qk(FL // 128)
                    emit_pair(0, 1, 0)
                    for st in range(4, NK):
                        emit_v(st)
                    for pr in range(1, HL // 2):
                        emit_qk(pr)
                        emit_qk(FL // 128 + pr)
                        emit_pair(2 * pr, 2 * pr + 1, 0)
                else:
                    for st in range(NK):
                        emit_v(st)
                    for fb in range(2 * FL // 128):
                        emit_qk(fb)
                    for pr in range(HL // 2):
                        emit_pair(2 * pr, 2 * pr + 1, 0)
                for qc in range(1, NQC):
                    for pr in range(HL // 2):
                        emit_pair(2 * pr, 2 * pr + 1, qc)
                    for qs in range(4):
                        emit_proj(4 * (qc - 1) + qs)
                    if reduce_mode == "rsc":
                        emit_rs_chunk(qc - 1)
                for qs in range(4):
                    emit_proj(4 * (NQC - 1) + qs)
                if reduce_mode == "rsc":
                    emit_rs_chunk(NQC - 1)

            if reduce_mode == "rs":
                nc.gpsimd.collective_compute(
                    "ReduceScatter",
                    mybir.AluOpType.add,
                    replica_groups=[[0, 1], [2, 3], [4, 5], [6, 7]],
                    ins=[rs_in.opt()],
                    outs=[rs_out.opt()],
                )
                nc.sync.dma_start(out=out_ext.ap(), in_=rs_out[:])

    nc.compile()
    return nc


@functools.lru_cache(maxsize=2)
def _built(cfg: str):
    return _build(cfg)


def _in_maps(x, c_attn_w, c_attn_b, c_proj_w, c_proj_b, reduce_mode):
    maps = []
    for core in range(N_CORES):
        b, hg = core // 2, core % 2
        f0 = hg * FL
        w_q = c_attn_w[:, f0:f0 + FL]
        w_k = c_attn_w[:, NX + f0:NX + f0 + FL]
        w_v = c_attn_w[:, 2 * NX + f0:2 * NX + f0 + FL]
        b_q = c_attn_b[f0:f0 + FL]
        b_k = c_attn_b[NX + f0:NX + f0 + FL]
        b_v = c_attn_b[2 * NX + f0:2 * NX + f0 + FL]
        maps.append({
            "xT": np.ascontiguousarray(x[b].T).astype(BF),
            "w_qk": np.concatenate([w_q, w_k], axis=1).astype(BF),
            "w_v": np.ascontiguousarray(w_v).astype(BF),
            "w_proj": np.ascontiguousarray(c_proj_w[f0:f0 + FL, :]).astype(BF),
            "b_qk": np.concatenate([b_q, b_k]).astype(np.float32).reshape(-1, 1),
            "bv_row": b_v.astype(BF).reshape(1, FL),
            "bp_row": (c_proj_b / 2.0).astype(BF).reshape(1, NX),
        })
    return maps


def _run(inputs, cfg=None, trace=False):
    if cfg is None:
        zero_bias = (not inputs["c_attn_b"].any()) and \
                    (not inputs["c_proj_b"].any())
        cfg = DEFAULT_CFG if zero_bias else DEFAULT_CFG_BIAS
    nc = _built(cfg)
    reduce_mode = _parse_cfg(cfg)["mode"]
    maps = _in_maps(inputs["x"], inputs["c_attn_w"], inputs["c_attn_b"],
                    inputs["c_proj_w"], inputs["c_proj_b"], reduce_mode)
    res = run_bass_kernel_spmd(nc, maps, core_ids=list(range(N_CORES)),
                               trace=trace)
    out = np.empty((B, S, NX), dtype=np.float32)
    for b in range(B):
        if reduce_mode == "rs":
            out[b, :S // 2] = res.results[2 * b]["out"]
            out[b, S // 2:] = res.results[2 * b + 1]["out"]
        elif reduce_mode == "rsc":
            o0 = res.results[2 * b]["out"].reshape(4, 256, NX)
            o1 = res.results[2 * b + 1]["out"].reshape(4, 256, NX)
            out[b] = np.concatenate([o0, o1], axis=1).reshape(S, NX)
        else:
            out[b] = res.results[2 * b]["out"] + res.results[2 * b + 1]["out"]
    return out, res


def kernel(**inputs):
    out, _ = _run({k: np.asarray(v) for k, v in inputs.items()})
    return out
